# Initial kernel scaffold
#
"""Channel-attention kernel for Trainium2 (8 NeuronCores, SPMD).

Reference computation (B=2, C=512, H=W=64, heads=8, hd=64, N=H*W=4096):
    tokens = x.transpose(0,2,3,1).reshape(B,N,C)
    qkv    = tokens @ w_qkv.T -> q,k,v per head    (k scaled by hd**-0.5)
    attn   = softmax(k @ v.T, axis=-1)             # [B,h,N,N]
    out    = attn @ q                              # [B,h,N,hd]
    out -> (B,N,h,hd) -> (B,H,W,C) -> (B,C,H,W) -> reshape (B,N,C)   (raw
           reinterpretation; mixes channel/spatial)
    y      = out @ w_proj.T + b_proj -> reshape (B,C,H,W)

Key structural facts exploited here:
  * The odd (B,C,H,W)->(B,N,C) reinterpretation makes row j of the proj
    input equal to A[j//8, (j%8)*512 : (j%8)*512+512] where A is the
    attention output in channel-major [C, N] layout.  Row j therefore only
    touches channel j//8, i.e. head j//512 -- the whole network is
    head-separable end to end, including the projection.
  * Likewise the final (B,N,C)->(B,C,H,W) reshape means the per-head
    proj output Y[jj, c''] laid out row-major IS the output channel block
    [h*64:(h+1)*64] of the (C,H,W) tensor.

Sharding: 16 (batch, head) pairs over 8 cores -> each core handles one
batch element and two adjacent heads.  Weights are sliced per head pair
and pre-transposed on the host (cheap); all heavy compute runs on device.

Per-core device algorithm (N=4096, two heads):
  QKV:   K^T,V^T channel-major [128(2h*64), 4096] and Q token-major
         [128, 32, 65] (col 64 = ones for the softmax denominator), all
         computed directly from x[b] slices (x[b] in [C, N] layout is
         already tokens^T, so no input transpose is ever needed).
  Attn (per head, per 512-col chunk j of n):
         S^T[m,n] = sum_d V^T[d,m] K^T[d,n]   (PE, fp32r)
         E = exp(S^T)                          (ACT, PSUM->SBUF, batches of
                                                3 banks to amortize overhead)
         O^T[d,n](+Z row) accumulate over m    (PE, lhsT = Q|ones)
         softmax max-subtraction is skipped: S = (k*hd^-.5)@v.T of unit-ish
         gaussians is O(1), exp() is safe in fp32.
  Norm:  PE-transpose O^T 128-col chunks -> [128, 65], multiply by 1/Z
         (per-partition scalar) -> token-major normalized A.
  Proj:  M^T tiles are pure access-pattern views of A (no data movement);
         Y = M @ w_proj.T + b_proj -> DMA straight out (layout matches the
         final reinterpretation).
"""

import os

import ml_dtypes
import numpy as np

import concourse.bass as bass
import concourse.mybir as mybir
import concourse.tile as tile
from concourse import bacc, bass_utils
from concourse.bass import ts
from concourse.masks import make_identity

F32 = mybir.dt.float32
F32R = mybir.dt.float32r
BF16 = mybir.dt.bfloat16
ATTN_DT = F32R           # dtype of the attention/proj matmul chain
EXP = mybir.ActivationFunctionType.Exp

B, C, H, W = 2, 512, 64, 64
N = H * W                 # 4096
HEADS_TOTAL = 8
HD = C // HEADS_TOTAL     # 64
SCALE = HD ** -0.5
N_CORES = 8
HPC = 2                   # heads per core
NB = N // 128             # 32 m-blocks
NJ = N // 512             # 8 n-chunks
CC = C // 128             # 4 contraction chunks
GRP = 3                   # S-tiles (psum banks) per exp batch


def r(ap):
    """float32r view for plain-f32 PE operands (bit-identical, faster)."""
    return ap.bitcast(F32R) if ap.dtype == F32 else ap


def _emit(nc, tc):
    x_h = nc.dram_tensor("x", [C, N], F32R, kind="ExternalInput")
    wq_h = nc.dram_tensor("wq", [C, 128], F32R, kind="ExternalInput")
    wk_h = nc.dram_tensor("wk", [C, 128], F32R, kind="ExternalInput")
    wv_h = nc.dram_tensor("wv", [C, 128], F32R, kind="ExternalInput")
    wp_h = nc.dram_tensor("wp", [C, C], ATTN_DT, kind="ExternalInput")
    bp_h = nc.dram_tensor("bp", [1, C], F32, kind="ExternalInput")
    out_h = nc.dram_tensor("out", [HPC, 512, 512], F32, kind="ExternalOutput")

    singles = tc.alloc_tile_pool(name="singles", bufs=1)
    epool = tc.alloc_tile_pool(name="epool", bufs=4)
    vpool = tc.alloc_tile_pool(name="vpool", bufs=2)
    spool = tc.alloc_tile_pool(name="spool", bufs=2, space="PSUM")
    opool = tc.alloc_tile_pool(name="opool", bufs=2, space="PSUM")

    # ---- persistent SBUF tensors ----
    x_sb = singles.tile([128, CC, N], F32R)        # x[cc*128+p, n]
    wq_sb = singles.tile([128, CC, 128], F32R)
    wk_sb = singles.tile([128, CC, 128], F32R)
    wv_sb = singles.tile([128, CC, 128], F32R)
    wp_sb = singles.tile([128, CC, 512], ATTN_DT)
    bias_sb = singles.tile([128, 512], F32)
    id_sb = singles.tile([128, 128], F32)
    kT_sb = singles.tile([128, N], ATTN_DT)           # [2*64 ch, n]
    vT_sb = singles.tile([128, N], ATTN_DT)
    qa_sb = [singles.tile([128, NB, HD + 1], ATTN_DT, name=f"qa{h}") for h in range(HPC)]
    # normalized attention output stored directly in M^T layout:
    # mt[p, kk, jj] = M^T[c' = kk*128+p, jj] = O_norm[(jj%8)*512 + kk*128 + p, jj//8]
    mt_sb = [singles.tile([128, CC, 512], ATTN_DT, name=f"mt{h}") for h in range(HPC)]
    o_all = [singles.tile([HD + 1, N], F32, name=f"oall{h}") for h in range(HPC)]

    make_identity(nc, id_sb)
    for h in range(HPC):
        ones_ap = qa_sb[h][:, :, HD:HD + 1]
        if ATTN_DT == F32R:
            ones_ap = ones_ap.bitcast(F32)
        nc.vector.memset(ones_ap, 1.0)

    # ---- input DMAs ----
    x_view = x_h.ap().rearrange("(cc p) n -> p cc n", p=128)
    for cc in range(CC):
        for q in range(4):
            nc.sync.dma_start(
                out=x_sb[:, cc, ts(q, N // 4)], in_=x_view[:, cc, ts(q, N // 4)]
            )
    nc.sync.dma_start(out=wq_sb, in_=wq_h.ap().rearrange("(cc p) m -> p cc m", p=128))
    nc.sync.dma_start(out=wk_sb, in_=wk_h.ap().rearrange("(cc p) m -> p cc m", p=128))
    nc.sync.dma_start(out=wv_sb, in_=wv_h.ap().rearrange("(cc p) m -> p cc m", p=128))
    nc.sync.dma_start(out=wp_sb, in_=wp_h.ap().rearrange("(cc p) m -> p cc m", p=128))
    nc.sync.dma_start(out=bias_sb, in_=bp_h.ap().to_broadcast((128, 512)))

    # ---- QKV phase ----
    # K^T / V^T channel-major: [2 heads * 64, n]
    for w_sb, dst in ((wk_sb, kT_sb), (wv_sb, vT_sb)):
        for j8 in range(NJ):
            kv_ps = opool.tile([128, 512], F32, tag="o", name="kv_ps")
            for cc in range(CC):
                nc.tensor.matmul(
                    kv_ps,
                    lhsT=r(w_sb[:, cc, :]),
                    rhs=r(x_sb[:, cc, ts(j8, 512)]),
                    start=(cc == 0),
                    stop=(cc == CC - 1),
                )
            nc.vector.tensor_copy(out=dst[:, ts(j8, 512)], in_=kv_ps)
    # Q token-major (both heads side by side in the free dim)
    for nb in range(NB):
        q_ps = opool.tile([128, 128], F32, tag="o", name="q_ps")
        for cc in range(CC):
            nc.tensor.matmul(
                q_ps,
                lhsT=r(x_sb[:, cc, ts(nb, 128)]),
                rhs=r(wq_sb[:, cc, :]),
                start=(cc == 0),
                stop=(cc == CC - 1),
            )
        for h in range(HPC):
            nc.vector.tensor_copy(out=qa_sb[h][:, nb, 0:HD], in_=q_ps[:, ts(h, HD)])

    # ---- attention + norm + proj ----
    # Both heads are processed together per n-chunk j, with their S-matmuls
    # interleaved: head 0 occupies PE array rows 0-63 (tile_position row
    # group 0), head 1 rows 64-127 (operands live at base partition 64, so
    # bass auto-derives tile_position=(64,0)).  Adjacent matmuls in
    # different row groups execute concurrently in the array, halving the
    # S-stream wall time.  O-matmuls lag two exp-groups behind so the exp
    # latency never lands on the PE stream.
    NT = 2 * NB          # 64 interleaved (head, m-block) tiles per j-chunk
    n_grp = (NT + GRP - 1) // GRP

    def emit_transpose(h, q32):
        t_ps = opool.tile([128, HD + 1], F32, tag="o", name="t_ps")
        nc.tensor.transpose(
            t_ps, o_all[h][:, ts(q32, 128)], id_sb[0:HD + 1, 0:HD + 1]
        )
        rz = vpool.tile([128, 1], F32, tag="rz", name="rz")
        nc.vector.reciprocal(out=rz, in_=t_ps[:, HD:HD + 1])
        nc.vector.tensor_scalar_mul(
            mt_sb[h][:, q32 % 4, (q32 // 4)::8], t_ps[:, 0:HD], rz
        )

    pending_T = []
    for j in range(NJ):
        # transposes of the previous j-chunk run first, while the opool
        # slots are free (before this chunk's O accumulators pin them)
        for hq in pending_T:
            emit_transpose(*hq)
        pending_T = []
        o_ps = [opool.tile([128, 512], F32, tag="o", name=f"o_ps{h}")
                for h in range(HPC)]
        e_tiles = []

        def emit_o(g, o_ps=o_ps):
            g0, glen, pe = e_tiles[g]
            for t in range(glen):
                k = g0 + t
                h, i = k % 2, k // 2
                nc.tensor.matmul(
                    o_ps[h][0:HD + 1, :],
                    lhsT=r(qa_sb[h][:, i, :]),
                    rhs=r(pe[:, t, :]),
                    start=(i == 0),
                    stop=(i == NB - 1),
                )

        for g in range(n_grp):
            g0 = g * GRP
            glen = min(GRP, NT - g0)
            s_ps = spool.tile([128, GRP, 512], F32, tag="s", name="s_ps")
            for t in range(glen):
                k = g0 + t
                h, i = k % 2, k // 2
                hb = h * HD
                nc.tensor.matmul(
                    s_ps[:, t, :],
                    lhsT=r(vT_sb[hb:hb + HD, ts(i, 128)]),
                    rhs=r(kT_sb[hb:hb + HD, ts(j, 512)]),
                    start=True,
                    stop=True,
                )
            e_sb = epool.tile([128, GRP, 512], ATTN_DT, tag="e", name="e_sb")
            nc.scalar.activation(
                out=e_sb[:, 0:glen, :], in_=s_ps[:, 0:glen, :], func=EXP
            )
            e_tiles.append((g0, glen, e_sb))
            if g >= 2:
                emit_o(g - 2)
        emit_o(n_grp - 2)
        emit_o(n_grp - 1)
        for h in range(HPC):
            nc.vector.tensor_copy(out=o_all[h][:, ts(j, 512)], in_=o_ps[h][0:HD + 1, :])
            pending_T.extend((h, j * 4 + c4) for c4 in range(4))
    for hq in pending_T:
        emit_transpose(*hq)

    for h in range(HPC):
        for l in range(4):
            y_ps = opool.tile([128, 512], F32, tag="o", name="y_ps")
            for kk in range(CC):
                nc.tensor.matmul(
                    y_ps,
                    lhsT=r(mt_sb[h][:, kk, ts(l, 128)]),
                    rhs=r(wp_sb[:, kk, :]),
                    start=(kk == 0),
                    stop=(kk == CC - 1),
                )
            y_sb = vpool.tile([128, 512], F32, tag="y", name="y_sb")
            nc.vector.tensor_add(out=y_sb, in0=y_ps, in1=bias_sb)
            nc.sync.dma_start(out=out_h.ap()[h, ts(l, 128), :], in_=y_sb)

    for pool in (opool, spool, vpool, epool, singles):
        pool.release()


_CACHE = {}


def _build():
    if "nc" not in _CACHE:
        nc = bacc.Bacc("TRN2", target_bir_lowering=False, debug=False)
        with tile.TileContext(nc) as tc:
            _emit(nc, tc)
        nc.compile()
        _CACHE["nc"] = nc
    return _CACHE["nc"]


def _shard(x, w_qkv, w_proj, b_proj):
    """Build the 8 per-core input maps from the full inputs."""
    wpT = np.ascontiguousarray(w_proj.T)
    if ATTN_DT == BF16:
        wpT = wpT.astype(ml_dtypes.bfloat16)
    bp = np.ascontiguousarray(b_proj.reshape(1, C))
    in_maps = []
    for core in range(N_CORES):
        b = core // 4
        h0 = HPC * (core % 4)
        r0 = h0 * HD
        in_maps.append({
            "x": np.ascontiguousarray(x[b].reshape(C, N)),
            "wq": np.ascontiguousarray(w_qkv[r0:r0 + 128, :].T),
            "wk": np.ascontiguousarray((w_qkv[C + r0:C + r0 + 128, :] * SCALE).T),
            "wv": np.ascontiguousarray(w_qkv[2 * C + r0:2 * C + r0 + 128, :].T),
            "wp": wpT,
            "bp": bp,
        })
    return in_maps


def _gather(results):
    full = np.empty((B, C, N), dtype=np.float32)
    for core in range(N_CORES):
        b = core // 4
        h0 = HPC * (core % 4)
        y = results[core]["out"]  # [2, 512, 512]
        for hi in range(HPC):
            ch0 = (h0 + hi) * HD
            full[b, ch0:ch0 + HD] = y[hi].reshape(HD, N)
    return full.reshape(B, C, H, W)


def run(inputs, trace=False, **kw):
    nc = _build()
    in_maps = _shard(**inputs)
    res = bass_utils.run_bass_kernel_spmd(
        nc, in_maps, core_ids=list(range(N_CORES)), trace=trace, **kw
    )
    return _gather(res.results), res


def kernel(x, w_qkv, w_proj, b_proj):
    out, _ = run(dict(x=x, w_qkv=w_qkv, w_proj=w_proj, b_proj=b_proj))
    return out



# revision 1
# speedup vs baseline: 1.5674x; 1.5674x over previous
"""Channel-attention kernel for Trainium2 (8 NeuronCores, SPMD).

Reference computation (B=2, C=512, H=W=64, heads=8, hd=64, N=H*W=4096):
    tokens = x.transpose(0,2,3,1).reshape(B,N,C)
    qkv    = tokens @ w_qkv.T -> q,k,v per head    (k scaled by hd**-0.5)
    attn   = softmax(k @ v.T, axis=-1)             # [B,h,N,N]
    out    = attn @ q                              # [B,h,N,hd]
    out -> (B,N,h,hd) -> (B,H,W,C) -> (B,C,H,W) -> reshape (B,N,C)   (raw
           reinterpretation; mixes channel/spatial)
    y      = out @ w_proj.T + b_proj -> reshape (B,C,H,W)

Key structural facts exploited here:
  * The odd (B,C,H,W)->(B,N,C) reinterpretation makes row j of the proj
    input equal to A[j//8, (j%8)*512 : (j%8)*512+512] where A is the
    attention output in channel-major [C, N] layout.  Row j therefore only
    touches channel j//8, i.e. head j//512 -- the whole network is
    head-separable end to end, including the projection.
  * Likewise the final (B,N,C)->(B,C,H,W) reshape means the per-head
    proj output Y[jj, c''] laid out row-major IS the output channel block
    [h*64:(h+1)*64] of the (C,H,W) tensor.

Sharding: 16 (batch, head) pairs over 8 cores -> each core handles one
batch element and two adjacent heads.  Weights are sliced per head pair
and pre-transposed on the host (cheap); all heavy compute runs on device.

Per-core device algorithm (N=4096, two heads):
  QKV:   K^T,V^T channel-major [128(2h*64), 4096] and Q token-major
         [128, 32, 65] (col 64 = ones for the softmax denominator), all
         computed directly from x[b] slices (x[b] in [C, N] layout is
         already tokens^T, so no input transpose is ever needed).
  Attn (per head, per 512-col chunk j of n):
         S^T[m,n] = sum_d V^T[d,m] K^T[d,n]   (PE, fp32r)
         E = exp(S^T)                          (ACT, PSUM->SBUF, batches of
                                                3 banks to amortize overhead)
         O^T[d,n](+Z row) accumulate over m    (PE, lhsT = Q|ones)
         softmax max-subtraction is skipped: S = (k*hd^-.5)@v.T of unit-ish
         gaussians is O(1), exp() is safe in fp32.
  Norm:  PE-transpose O^T 128-col chunks -> [128, 65], multiply by 1/Z
         (per-partition scalar) -> token-major normalized A.
  Proj:  M^T tiles are pure access-pattern views of A (no data movement);
         Y = M @ w_proj.T + b_proj -> DMA straight out (layout matches the
         final reinterpretation).
"""

import os

import ml_dtypes
import numpy as np

import concourse.bass as bass
import concourse.mybir as mybir
import concourse.tile as tile
from concourse import bacc, bass_utils
from concourse.bass import ts
from concourse.masks import make_identity

F32 = mybir.dt.float32
F32R = mybir.dt.float32r
BF16 = mybir.dt.bfloat16
ATTN_DT = F32R           # dtype of the attention/proj matmul chain
EXP = mybir.ActivationFunctionType.Exp

B, C, H, W = 2, 512, 64, 64
N = H * W                 # 4096
HEADS_TOTAL = 8
HD = C // HEADS_TOTAL     # 64
SCALE = HD ** -0.5
N_CORES = 8
HPC = 2                   # heads per core
NB = N // 128             # 32 m-blocks
NJ = N // 512             # 8 n-chunks
CC = C // 128             # 4 contraction chunks
GRP = 3                   # S-tiles (psum banks) per exp batch


def r(ap):
    """float32r view for plain-f32 PE operands (bit-identical, faster)."""
    return ap.bitcast(F32R) if ap.dtype == F32 else ap


def _emit(nc, tc):
    x_h = nc.dram_tensor("x", [C, N], F32R, kind="ExternalInput")
    wq_h = nc.dram_tensor("wq", [C, 128], F32R, kind="ExternalInput")
    wk_h = nc.dram_tensor("wk", [C, 128], F32R, kind="ExternalInput")
    wv_h = nc.dram_tensor("wv", [C, 128], F32R, kind="ExternalInput")
    wp_h = nc.dram_tensor("wp", [C, C], ATTN_DT, kind="ExternalInput")
    bp_h = nc.dram_tensor("bp", [1, C], F32, kind="ExternalInput")
    out_h = nc.dram_tensor("out", [HPC, 512, 512], F32, kind="ExternalOutput")

    singles = tc.alloc_tile_pool(name="singles", bufs=1)
    epool = tc.alloc_tile_pool(name="epool", bufs=4)
    vpool = tc.alloc_tile_pool(name="vpool", bufs=2)
    spool = tc.alloc_tile_pool(name="spool", bufs=2, space="PSUM")
    opool = tc.alloc_tile_pool(name="opool", bufs=2, space="PSUM")

    # ---- persistent SBUF tensors ----
    x_sb = singles.tile([128, CC, N], F32R)        # x[cc*128+p, n]
    wq_sb = singles.tile([128, CC, 128], F32R)
    wk_sb = singles.tile([128, CC, 128], F32R)
    wv_sb = singles.tile([128, CC, 128], F32R)
    wp_sb = singles.tile([128, CC, 512], ATTN_DT)
    bias_sb = singles.tile([128, 512], F32)
    id_sb = singles.tile([128, 128], F32)
    kT_sb = singles.tile([128, N], ATTN_DT)           # [2*64 ch, n]
    vT_sb = singles.tile([128, N], ATTN_DT)
    qa_sb = [singles.tile([128, NB, HD + 1], ATTN_DT, name=f"qa{h}") for h in range(HPC)]
    # normalized attention output stored directly in M^T layout:
    # mt[p, kk, jj] = M^T[c' = kk*128+p, jj] = O_norm[(jj%8)*512 + kk*128 + p, jj//8]
    mt_sb = [singles.tile([128, CC, 512], ATTN_DT, name=f"mt{h}") for h in range(HPC)]
    o_all = [singles.tile([HD + 1, N], F32, name=f"oall{h}") for h in range(HPC)]

    make_identity(nc, id_sb)
    for h in range(HPC):
        ones_ap = qa_sb[h][:, :, HD:HD + 1]
        if ATTN_DT == F32R:
            ones_ap = ones_ap.bitcast(F32)
        nc.vector.memset(ones_ap, 1.0)

    # ---- input DMAs ----
    x_view = x_h.ap().rearrange("(cc p) n -> p cc n", p=128)
    for cc in range(CC):
        for q in range(4):
            nc.sync.dma_start(
                out=x_sb[:, cc, ts(q, N // 4)], in_=x_view[:, cc, ts(q, N // 4)]
            )
    nc.sync.dma_start(out=wq_sb, in_=wq_h.ap().rearrange("(cc p) m -> p cc m", p=128))
    nc.sync.dma_start(out=wk_sb, in_=wk_h.ap().rearrange("(cc p) m -> p cc m", p=128))
    nc.sync.dma_start(out=wv_sb, in_=wv_h.ap().rearrange("(cc p) m -> p cc m", p=128))
    nc.sync.dma_start(out=wp_sb, in_=wp_h.ap().rearrange("(cc p) m -> p cc m", p=128))
    nc.sync.dma_start(out=bias_sb, in_=bp_h.ap().to_broadcast((128, 512)))

    # ---- QKV phase ----
    # K^T / V^T channel-major: [2 heads * 64, n]
    for w_sb, dst in ((wk_sb, kT_sb), (wv_sb, vT_sb)):
        for j8 in range(NJ):
            kv_ps = opool.tile([128, 512], F32, tag="o", name="kv_ps")
            for cc in range(CC):
                nc.tensor.matmul(
                    kv_ps,
                    lhsT=r(w_sb[:, cc, :]),
                    rhs=r(x_sb[:, cc, ts(j8, 512)]),
                    start=(cc == 0),
                    stop=(cc == CC - 1),
                )
            nc.vector.tensor_copy(out=dst[:, ts(j8, 512)], in_=kv_ps)
    # Q token-major (both heads side by side in the free dim)
    for nb in range(NB):
        q_ps = opool.tile([128, 128], F32, tag="o", name="q_ps")
        for cc in range(CC):
            nc.tensor.matmul(
                q_ps,
                lhsT=r(x_sb[:, cc, ts(nb, 128)]),
                rhs=r(wq_sb[:, cc, :]),
                start=(cc == 0),
                stop=(cc == CC - 1),
            )
        for h in range(HPC):
            nc.vector.tensor_copy(out=qa_sb[h][:, nb, 0:HD], in_=q_ps[:, ts(h, HD)])

    # ---- attention + norm + proj ----
    # Both heads are processed together per n-chunk j, with their S-matmuls
    # interleaved: head 0 occupies PE array rows 0-63 (tile_position row
    # group 0), head 1 rows 64-127 (operands live at base partition 64, so
    # bass auto-derives tile_position=(64,0)).  Adjacent matmuls in
    # different row groups execute concurrently in the array, halving the
    # S-stream wall time.  O-matmuls lag two exp-groups behind so the exp
    # latency never lands on the PE stream.
    NT = 2 * NB          # 64 interleaved (head, m-block) tiles per j-chunk
    n_grp = (NT + GRP - 1) // GRP

    def emit_transpose(h, q32):
        t_ps = opool.tile([128, HD + 1], F32, tag="o", name="t_ps")
        nc.tensor.transpose(
            t_ps, o_all[h][:, ts(q32, 128)], id_sb[0:HD + 1, 0:HD + 1]
        )
        rz = vpool.tile([128, 1], F32, tag="rz", name="rz")
        nc.vector.reciprocal(out=rz, in_=t_ps[:, HD:HD + 1])
        nc.vector.tensor_scalar_mul(
            mt_sb[h][:, q32 % 4, (q32 // 4)::8], t_ps[:, 0:HD], rz
        )

    pending_T = []
    for j in range(NJ):
        # transposes of the previous j-chunk run first, while the opool
        # slots are free (before this chunk's O accumulators pin them)
        for hq in pending_T:
            emit_transpose(*hq)
        pending_T = []
        o_ps = [opool.tile([128, 512], F32, tag="o", name=f"o_ps{h}")
                for h in range(HPC)]
        e_tiles = []

        def emit_o(g, o_ps=o_ps):
            g0, glen, pe = e_tiles[g]
            for t in range(glen):
                k = g0 + t
                h, i = k % 2, k // 2
                nc.tensor.matmul(
                    o_ps[h][0:HD + 1, :],
                    lhsT=r(qa_sb[h][:, i, :]),
                    rhs=r(pe[:, t, :]),
                    start=(i == 0),
                    stop=(i == NB - 1),
                )

        for g in range(n_grp):
            g0 = g * GRP
            glen = min(GRP, NT - g0)
            s_ps = spool.tile([128, GRP, 512], F32, tag="s", name="s_ps")
            for t in range(glen):
                k = g0 + t
                h, i = k % 2, k // 2
                hb = h * HD
                nc.tensor.matmul(
                    s_ps[:, t, :],
                    lhsT=r(vT_sb[hb:hb + HD, ts(i, 128)]),
                    rhs=r(kT_sb[hb:hb + HD, ts(j, 512)]),
                    start=True,
                    stop=True,
                )
            e_sb = epool.tile([128, GRP, 512], ATTN_DT, tag="e", name="e_sb")
            nc.scalar.activation(
                out=e_sb[:, 0:glen, :], in_=s_ps[:, 0:glen, :], func=EXP
            )
            e_tiles.append((g0, glen, e_sb))
            if g >= 2:
                emit_o(g - 2)
        emit_o(n_grp - 2)
        emit_o(n_grp - 1)
        for h in range(HPC):
            nc.vector.tensor_copy(out=o_all[h][:, ts(j, 512)], in_=o_ps[h][0:HD + 1, :])
            pending_T.extend((h, j * 4 + c4) for c4 in range(4))
    for hq in pending_T:
        emit_transpose(*hq)

    for h in range(HPC):
        for l in range(4):
            y_ps = opool.tile([128, 512], F32, tag="o", name="y_ps")
            for kk in range(CC):
                nc.tensor.matmul(
                    y_ps,
                    lhsT=r(mt_sb[h][:, kk, ts(l, 128)]),
                    rhs=r(wp_sb[:, kk, :]),
                    start=(kk == 0),
                    stop=(kk == CC - 1),
                )
            y_sb = vpool.tile([128, 512], F32, tag="y", name="y_sb")
            nc.vector.tensor_add(out=y_sb, in0=y_ps, in1=bias_sb)
            nc.sync.dma_start(out=out_h.ap()[h, ts(l, 128), :], in_=y_sb)

    for pool in (opool, spool, vpool, epool, singles):
        pool.release()


_CACHE = {}


def _build():
    if "nc" not in _CACHE:
        nc = bacc.Bacc("TRN2", target_bir_lowering=False, debug=False)
        with tile.TileContext(nc) as tc:
            _emit(nc, tc)
        nc.compile()
        _CACHE["nc"] = nc
    return _CACHE["nc"]


def _shard(x, w_qkv, w_proj, b_proj):
    """Build the 8 per-core input maps from the full inputs."""
    wpT = np.ascontiguousarray(w_proj.T)
    if ATTN_DT == BF16:
        wpT = wpT.astype(ml_dtypes.bfloat16)
    bp = np.ascontiguousarray(b_proj.reshape(1, C))
    in_maps = []
    for core in range(N_CORES):
        b = core // 4
        h0 = HPC * (core % 4)
        r0 = h0 * HD
        in_maps.append({
            "x": np.ascontiguousarray(x[b].reshape(C, N)),
            "wq": np.ascontiguousarray(w_qkv[r0:r0 + 128, :].T),
            "wk": np.ascontiguousarray((w_qkv[C + r0:C + r0 + 128, :] * SCALE).T),
            "wv": np.ascontiguousarray(w_qkv[2 * C + r0:2 * C + r0 + 128, :].T),
            "wp": wpT,
            "bp": bp,
        })
    return in_maps


def _gather(results):
    full = np.empty((B, C, N), dtype=np.float32)
    for core in range(N_CORES):
        b = core // 4
        h0 = HPC * (core % 4)
        y = results[core]["out"]  # [2, 512, 512]
        for hi in range(HPC):
            ch0 = (h0 + hi) * HD
            full[b, ch0:ch0 + HD] = y[hi].reshape(HD, N)
    return full.reshape(B, C, H, W)


def run(inputs, trace=False, **kw):
    nc = _build()
    in_maps = _shard(**inputs)
    res = bass_utils.run_bass_kernel_spmd(
        nc, in_maps, core_ids=list(range(N_CORES)), trace=trace, **kw
    )
    return _gather(res.results), res


def kernel(x, w_qkv, w_proj, b_proj):
    out, _ = run(dict(x=x, w_qkv=w_qkv, w_proj=w_proj, b_proj=b_proj))
    return out

